# revision 21
# baseline (speedup 1.0000x reference)
"""Trainium2 Bass kernel for nn_Actor_39307540693502 (gnn_message_passing).

Data-parallel over the 8 graphs (one graph per NeuronCore, no collectives).
Per core:

  Vg = (A@V)@W_n2n1 + b_n2n1 + (B@E)@W_e2n1 + b_e2n1     [1024, 10]
  Eg = (B.T@V)@W_n2e1 + b_n2e1 + E@W_e2e1 + b_e2e1       [16384, 10]
  V1 = relu(Vg); E1 = relu(Eg)
  out = (A@V1)@W_n2n3 + b_n2n3 + (B@E1)@W_e2n3 + b_e2n3  [1024, 2]

B (64 MB f32 per core) dominates. The TensorEngine contracts over the
partition axis of both operands, so the N-contraction (B.T@V) and the
M-contractions (B@E, B@E1) need B in SBUF with different partition axes;
on-chip bulk transposes are slower than re-reading, so the host supplies
B in both layouts, both cast to fp8-e4m3 (2 x 16 MB per core, contiguous
4 MB DMAs with 32 KB per-partition runs). Accuracy holds because fp8
noise on the high-dimensional operands washes out as sqrt(M) inside the
16384/1024-term contractions; the low-dimensional V (whose quantization
error would correlate across all edges) is fed as fp8(V) plus an fp8
residual correction, and the Eg head sums both halves for free. Both big
streaming passes run as fp8 DoubleRow matmuls (2 m-slabs per
instruction). Everything else (Eg/V1/output heads) folds into small
stacked matmuls with biases applied via a ones-row; E1 is computed
chunk-by-chunk in natural [m, 10] orientation so the single pass over
the transposed copy of B can both consume it and accumulate (B@E1).T.

Single full run on silicon ~110-135 us per core (~34 MB of HBM traffic
at the ~358 GB/s per-core limit, PE ~75 us hidden underneath).
"""

import numpy as np
import ml_dtypes

BF16 = ml_dtypes.bfloat16
FP8 = ml_dtypes.float8_e4m3
BSZ, N, M = 8, 1024, 16384
MC = 512                # edge columns per chunk
NCH = M // MC           # 32 chunks
TPC = MC // 128         # 4 m-slabs per chunk
NS = N // 128           # 8 node slabs
GSZ = 8                 # chunks per DMA group (4 MB contiguous window)
BUFS = 2                # group buffers per B-stream pool

_CACHE = {}


class _null_iter:
    """Context-manager-less stand-in matching tc.For_i's with-usage."""
    def __init__(self, it):
        self.it = it
    def __iter__(self):
        return iter(self.it)


def _install_tile_patch():
    """This container's walrus build accepts at most ONE sync-wait per
    instruction; Tile's kernel-tail drain attaches one wait per tracked
    semaphore lane. Split the extra waits onto single-wait NOPs on the
    same (sync) sequencer — executed in program order before the
    all-engine barrier, so semantically identical."""
    import bass_rust
    from concourse import tile as tile_mod
    from concourse.vector_clock import ScopedClock

    if getattr(tile_mod.TileContext, "_ant_drain_patched", False):
        return

    def _patched(self, tick_clock, wait_clock):
        drain_inst = self.nc.sync.drain()
        wait_clock.add_sem_waits(
            drain_inst.ins, ScopedClock({None: tick_clock.global_clock})
        )
        si = drain_inst.ins.sync_info
        waits = list(si.on_wait) if si is not None else []
        if len(waits) > 1:
            drain_inst.ins.sync_info = bass_rust.SyncInfo(
                on_wait=[waits[0]], on_update=list(si.on_update)
            )
            for w in waits[1:]:
                nop = self.nc.sync.nop()
                nop.ins.sync_info = bass_rust.SyncInfo(on_wait=[w], on_update=[])
        self.nc.all_engine_barrier()
        assert self.sems is not None
        popped = self.nc._tile_sem_poison_stack.pop()
        assert popped is self._sem_poison
        self.nc.clear_and_free_semaphores(list(self.sems.allocated().values()))
        self.nc.all_engine_barrier()

    tile_mod.TileContext._drain_and_barrier = _patched
    tile_mod.TileContext._ant_drain_patched = True


def _split_multi_waits(nc):
    """This walrus build accepts at most one sync-wait per instruction.
    Tile's scheduler attaches one wait per producer lane, so split the
    extras onto single-wait NOPs inserted just before, on the same engine
    queue (sequencer executes them in program order — same semantics)."""
    import bass_rust
    import concourse.mybir as mybir

    cnt = 0
    for f in nc.m.functions:
        for bb in f.blocks:
            new_insts = []
            changed = False
            for inst in bb.instructions:
                si = inst.sync_info
                waits = list(si.on_wait) if si is not None else []
                if len(waits) > 1:
                    changed = True
                    for w in waits[:-1]:
                        nop = mybir.InstNoOp(
                            name=f"{inst.name}-wsplit{cnt}", ins=[], outs=[]
                        )
                        cnt += 1
                        nop.engine = inst.engine
                        nop.sync_info = bass_rust.SyncInfo(
                            on_wait=[w], on_update=[]
                        )
                        new_insts.append(nop)
                    inst.sync_info = bass_rust.SyncInfo(
                        on_wait=[waits[-1]], on_update=list(si.on_update)
                    )
                new_insts.append(inst)
            if changed:
                bb.instructions = new_insts
    return cnt


def build_nc(reps=1, internal_inputs=False, loop_reps=False, dma_only=False,
             unroll=8):
    """reps>1 runs the whole computation that many times inside one NEFF
    (same inputs, same output) — used only for wall-clock timing, where
    per-dispatch overhead is differenced out. internal_inputs=True swaps
    the big input parameters for internal DRAM tensors (garbage values) so
    timing dispatches don't pay the per-call input copy."""
    _install_tile_patch()
    import concourse.bass as bass
    import concourse.mybir as mybir
    from concourse import tile

    bf = mybir.dt.bfloat16
    fp8 = mybir.dt.float8e4
    f32 = mybir.dt.float32
    ACT = mybir.ActivationFunctionType

    nc = bass.Bass()
    if internal_inputs:
        par = lambda name, shape, dt: nc.dram_tensor(name, shape, dt)
    else:
        par = lambda name, shape, dt: nc.declare_dram_parameter(
            name, shape, dt, isOutput=False
        )
    # Chunk-major in HBM: a group of chunks [g0:g0+sz] is one fully
    # contiguous window (sz x 512 KB), maximizing HBM row locality.
    Bn = par("Bn", [NCH, 128, NS, MC], fp8)
    Bt = par("Bt", [NCH, 128, TPC, N], fp8)
    At = par("At", [128, NS, N], bf)
    Vt = par("Vt", [128, NS, 2], bf)
    Vt8 = par("Vt8", [128, NS, 16], fp8)  # cols 0-1 fp8(V), 2-3 fp8 residual
    Et = par("Et", [128, NCH, TPC], fp8)
    Eo = par("Eo", [2, M], bf)
    Wp = par("Wp", [11, 56], bf)
    OUT = nc.declare_dram_parameter("out", [128, NS, 2], f32, isOutput=True)

    with tile.TileContext(nc) as tc:
        with (
            tc.tile_pool(name="consts", bufs=1) as consts,
            tc.tile_pool(name="ps_out", bufs=1, space="PSUM") as ps_out_pool,
            tc.tile_pool(name="bn", bufs=BUFS) as bnp,
            tc.tile_pool(name="bt", bufs=BUFS) as btp,
            tc.tile_pool(name="e1", bufs=6) as e1p,
            tc.tile_pool(name="ps_vtb", bufs=2, space="PSUM") as ps_vtb,
            tc.tile_pool(name="ps_eg", bufs=2, space="PSUM") as ps_eg,
            tc.tile_pool(name="ps_misc", bufs=1, space="PSUM") as misc,
        ):
            at_sb = consts.tile([128, NS, N], bf, tag="at")
            vt_sb = consts.tile([128, NS, 2], bf, tag="vt")
            vt8_sb = consts.tile([128, NS, 16], fp8, tag="vt8")
            et_sb = consts.tile([128, NCH, TPC], fp8, tag="et")
            st6 = consts.tile([6, M], bf, tag="st6")  # [v8tb(2); vres_tb(2); E; ones]
            wp_sb = consts.tile([11, 56], bf, tag="wp")
            ones_sb = consts.tile([1, N], bf, tag="ones")
            nc.scalar.dma_start(out=at_sb[:], in_=At[:])
            nc.sync.dma_start(out=vt_sb[:], in_=Vt[:])
            nc.sync.dma_start(out=vt8_sb[:], in_=Vt8[:])
            nc.sync.dma_start(out=et_sb[:], in_=Et[:])
            nc.sync.dma_start(out=st6[4:6, :], in_=Eo[:])
            nc.sync.dma_start(out=wp_sb[:], in_=Wp[:])
            nc.vector.memset(ones_sb[:], 1.0)

            # DMA group schedule. The first rep ramps up (1, 1, 2, then 4
            # chunks per group) so PE starts ~6 us into the kernel instead
            # of waiting for a full 4 MB pair of groups; steady-state reps
            # use uniform G-chunk groups (fewer DMA fixed costs).
            G = GSZ
            def group_map(ramp):
                head = [(0, 1), (1, 1), (2, 2)] if ramp else [(0, 4)]
                start = 4
                groups = head + [
                    (c, min(G, NCH - c)) for c in range(start, NCH, G)
                ]
                m = {}
                for g0, sz in groups:
                    for cc in range(g0, g0 + sz):
                        m[cc] = (g0, sz)
                return m
            # (A@V).T — rep-independent, runs up front under the B stream
            av_ps = misc.tile([2, N], f32, tag="misc")
            for s in range(NS):
                for h in range(2):
                    nc.tensor.matmul(
                        av_ps[:, h * 512 : (h + 1) * 512],
                        vt_sb[:, s, :],
                        at_sb[:, s, h * 512 : (h + 1) * 512],
                        start=(s == 0),
                        stop=(s == NS - 1),
                    )
            av_sb = consts.tile([2, N], bf, tag="av")
            nc.scalar.activation(av_sb[:], av_ps[:], ACT.Copy)

            def rep_body(ramp=False):
                CHUNK_GROUP = group_map(ramp)
                # row 0 = (B@E).T, rows 1..10 = (B@E1).T — over all chunks.
                # E sits at partition 0 so the V1 head can slice it directly
                # (engine APs must start at partition 0/32/64), killing the
                # per-rep SBUF->SBUF row-move DMA on the sync ring.
                po = ps_out_pool.tile([11, N], f32, tag="po")

                for c in range(NCH):
                    g0, sz = CHUNK_GROUP[c]
                    ci = c - g0
                    if ci == 0:
                        bng = bnp.tile([128, sz, NS, MC], fp8, tag="bn")
                        nc.sync.dma_start(
                            out=bng[:],
                            in_=Bn[g0 : g0 + sz].rearrange("c p s m -> p c s m"),
                        )
                        btg = btp.tile([128, sz, TPC, N], fp8, tag="bt")
                        nc.scalar.dma_start(
                            out=btg[:],
                            in_=Bt[g0 : g0 + sz].rearrange("c p t n -> p c t n"),
                        )
                    bn = bng[:, ci]
                    bt = btg[:, ci]
                    if dma_only:
                        continue

                    # P1 (fp8 DoubleRow over slab pairs):
                    # rows 0-1 = fp8(V).T B, rows 2-3 = fp8-residual(V).T B
                    vtb_ps = ps_vtb.tile([4, MC], f32, tag="vtb_ps")
                    for s2 in range(NS // 2):
                        nc.tensor.matmul(
                            vtb_ps[:],
                            vt8_sb[:, 2 * s2 : 2 * s2 + 2, 0:4],
                            bn[:, 2 * s2 : 2 * s2 + 2, :],
                            start=(s2 == 0),
                            stop=(s2 == NS // 2 - 1),
                            perf_mode=mybir.MatmulPerfMode.DoubleRow,
                        )
                    nc.vector.tensor_copy(
                        st6[0:4, c * MC : (c + 1) * MC], vtb_ps[:]
                    )

                    # Eg natural [m, 10] per m-slab, biases via Eo ones-row
                    eg_ps = ps_eg.tile([128, TPC, 10], f32, tag="eg_ps")
                    for t in range(TPC):
                        g0 = c * MC + t * 128
                        nc.tensor.matmul(
                            eg_ps[:, t, :],
                            st6[:, g0 : g0 + 128],
                            wp_sb[0:6, 36:46],
                            start=True,
                            stop=True,
                        )
                    e1 = e1p.tile([128, TPC, 16], fp8, tag="e1")
                    nc.vector.tensor_relu(e1[:, :, 1:11], eg_ps[:])
                    nc.vector.tensor_copy(e1[:, :, 0], et_sb[:, c, :])

                    # P2: po[0,n] += sum_m E[m] B[n,m]; po[1+h,n] += sum_m E1[m,h] B[n,m]
                    for tp in range(TPC // 2):
                        first = c == 0 and tp == 0
                        last = c == NCH - 1 and tp == TPC // 2 - 1
                        nc.tensor.matmul(
                            po[:, 0:512],
                            e1[:, 2 * tp : 2 * tp + 2, 0:11],
                            bt[:, 2 * tp : 2 * tp + 2, 0:512],
                            start=first,
                            stop=last,
                            perf_mode=mybir.MatmulPerfMode.DoubleRow,
                        )
                        nc.tensor.matmul(
                            po[:, 512:1024],
                            e1[:, 2 * tp : 2 * tp + 2, 0:11],
                            bt[:, 2 * tp : 2 * tp + 2, 512:1024],
                            start=first,
                            stop=last,
                            perf_mode=mybir.MatmulPerfMode.DoubleRow,
                        )

                if dma_only:
                    osb0 = consts.tile([128, NS, 2], f32, tag="osb")
                    nc.vector.memset(osb0[:], 0.0)
                    nc.sync.dma_start(out=OUT[:], in_=osb0[:])
                    return
                # ---- post phase: A-passes + heads ----
                # Engine APs must start at partition 0 (or 32/64), so heads
                # are built from multiple accumulating matmuls over
                # partition-0-based source tiles; the (B@E).T row is at
                # partition 0 of bet_all, sliceable directly.
                if True:
                    bet_all = consts.tile([11, N], bf, tag="bet")
                    nc.scalar.activation(bet_all[:], po[:], ACT.Copy)

                    # V1 natural [n, 10] = relu(AV@W_n2n1 + BE@W_e2n1 + b1sum)
                    v1_ps = misc.tile([128, NS, 10], f32, tag="misc")
                    for s in range(NS):
                        sl = slice(s * 128, (s + 1) * 128)
                        nc.tensor.matmul(
                            v1_ps[:, s, :], av_sb[:, sl], wp_sb[0:2, 6:16],
                            start=True, stop=False,
                        )
                        nc.tensor.matmul(
                            v1_ps[:, s, :], bet_all[0:1, sl], wp_sb[0:1, 16:26],
                            start=False, stop=False,
                        )
                        nc.tensor.matmul(
                            v1_ps[:, s, :], ones_sb[:, sl], wp_sb[0:1, 26:36],
                            start=False, stop=True,
                        )
                    v1 = consts.tile([128, NS, 10], bf, tag="v1")
                    nc.scalar.activation(v1[:], v1_ps[:], ACT.Relu)

                    # (A@V1).T
                    av1_ps = misc.tile([10, N], f32, tag="misc")
                    for s in range(NS):
                        for h in range(2):
                            nc.tensor.matmul(
                                av1_ps[:, h * 512 : (h + 1) * 512],
                                v1[:, s, :],
                                at_sb[:, s, h * 512 : (h + 1) * 512],
                                start=(s == 0),
                                stop=(s == NS - 1),
                            )
                    av1_sb = consts.tile([10, N], bf, tag="av1")
                    nc.scalar.activation(av1_sb[:], av1_ps[:], ACT.Copy)

                    # out natural [n, 2] = AV1@W_n2n3 + BE1@W_e2n3 + b3sum
                    fin_ps = misc.tile([128, NS, 2], f32, tag="misc")
                    for s in range(NS):
                        sl = slice(s * 128, (s + 1) * 128)
                        nc.tensor.matmul(
                            fin_ps[:, s, :], av1_sb[:, sl], wp_sb[0:10, 0:2],
                            start=True, stop=False,
                        )
                        nc.tensor.matmul(
                            fin_ps[:, s, :], bet_all[0:11, sl], wp_sb[0:11, 2:4],
                            start=False, stop=False,
                        )
                        nc.tensor.matmul(
                            fin_ps[:, s, :], ones_sb[:, sl], wp_sb[0:1, 4:6],
                            start=False, stop=True,
                        )
                    osb = consts.tile([128, NS, 2], f32, tag="osb")
                    nc.scalar.activation(osb[:], fin_ps[:], ACT.Copy)
                    nc.sync.dma_start(out=OUT[:], in_=osb[:])

            if loop_reps:
                # Unroll several reps per For_i iteration: the loop's
                # all-engine barrier + drained tail cost ~10us per
                # iteration, amortized over `u` reps.
                u = max(x for x in (unroll, 4, 2, 1) if reps % x == 0)
                with tc.For_i(0, reps // u, 1):
                    for _ in range(u):
                        rep_body(ramp=False)
            else:
                for i in range(reps):
                    rep_body(ramp=(i == 0))

    _split_multi_waits(nc)
    return nc


def prep_inputs(A, B, V, E,
                W_n2n1, b_n2n1, W_e2n1, b_e2n1,
                W_n2e1, b_n2e1, W_e2e1, b_e2e1,
                W_n2n3, b_n2n3, W_e2n3, b_e2n3):
    """Shard + lay out the full f32 inputs for the 8 cores (bf16)."""
    Wp = np.zeros((11, 56), np.float32)
    Wp[0:10, 0:2] = W_n2n3
    Wp[1:11, 2:4] = W_e2n3  # row 0 (the E row of bet_all) contributes 0
    Wp[0, 4:6] = b_n2n3 + b_e2n3
    Wp[0:2, 6:16] = W_n2n1
    Wp[0, 16:26] = W_e2n1[0]
    Wp[0, 26:36] = b_n2n1 + b_e2n1
    Wp[0:2, 36:46] = W_n2e1
    Wp[2:4, 36:46] = W_n2e1
    Wp[4, 36:46] = W_e2e1[0]
    Wp[5, 36:46] = b_n2e1 + b_e2e1
    Wp = Wp.astype(BF16)

    in_maps = []
    for g in range(BSZ):
        Bgf = np.asarray(B[g], np.float32)
        Bg = Bgf.astype(BF16)
        Bg8 = Bgf.astype(FP8)
        Bn = np.ascontiguousarray(
            Bg8.reshape(NS, 128, NCH, MC).transpose(2, 1, 0, 3)
        )
        Bt = np.ascontiguousarray(
            Bgf.T.reshape(NCH, TPC, 128, N).transpose(0, 2, 1, 3)
        ).astype(FP8)
        Ag = np.asarray(A[g], np.float32).astype(BF16)
        At = np.ascontiguousarray(Ag.T.reshape(NS, 128, N).transpose(1, 0, 2))
        Vgf = np.asarray(V[g], np.float32)
        Vg = Vgf.astype(BF16)
        Vt = np.ascontiguousarray(Vg.reshape(NS, 128, 2).transpose(1, 0, 2))
        V8 = Vgf.astype(FP8)
        Vres = (Vgf - V8.astype(np.float32)).astype(FP8)
        Vt8 = np.zeros((128, NS, 16), FP8)
        Vt8[:, :, 0:2] = V8.reshape(NS, 128, 2).transpose(1, 0, 2)
        Vt8[:, :, 2:4] = Vres.reshape(NS, 128, 2).transpose(1, 0, 2)
        Eg = np.asarray(E[g], np.float32)[:, 0]
        Et = np.ascontiguousarray(
            Eg.reshape(NCH, TPC, 128).transpose(2, 0, 1).astype(FP8)
        )
        Eo = np.stack([Eg, np.ones_like(Eg)]).astype(BF16)
        in_maps.append(
            {"Bn": Bn, "Bt": Bt, "At": At, "Vt": Vt, "Vt8": Vt8,
             "Et": Et, "Eo": Eo, "Wp": Wp}
        )
    return in_maps


def gather_out(results):
    outs = []
    for g in range(BSZ):
        o = np.asarray(results[g]["out"])  # [128, NS, 2]
        outs.append(o.transpose(1, 0, 2).reshape(N, 2))
    return np.stack(outs).astype(np.float32)


def kernel(**inputs):
    from concourse.bass_utils import run_bass_kernel_spmd

    if "nc" not in _CACHE:
        _CACHE["nc"] = build_nc()
    nc = _CACHE["nc"]
    in_maps = prep_inputs(**{k: np.asarray(v) for k, v in inputs.items()})
    res = run_bass_kernel_spmd(nc, in_maps, core_ids=list(range(BSZ)))
    return gather_out(res.results)

